# revision 38
# baseline (speedup 1.0000x reference)
"""Trainium2 Bass kernel for nn_PlatonicConv (linear-attention GNN message passing).

Math (reference):
  q = rope(x@Wq + bq, phase);  k = rope(ones, phase);  v = x@Wv + bv
  phase[n, g, p] = pos[n, :] . freqs[g, 0, p, :]
  KV_b[g] = (1/AVG) * sum_{n in graph b} k[n,g,:] (x) v[n,g,:]
  out[n]  = concat_g( q'[n,g,:] @ KV_b[g] ) @ Wo + bo

Device formulation (per core, data-parallel over graphs; 8 graphs/core):
  host precomputes cos/sin of phase (feature-major, bf16) and node-major
  k = rope(ones) (unscaled; 1/AVG folded into Wo).  Per graph b:
    M_b = stack_rows(KV_b[g] @ Wo[g-rows]) : [384, 384]
    out[n] = q'[n] @ M_{b(n)}  (+ bo on host).
  q'/M_b rows use "A-order" over rope pairs t = g*16+p:
    rows   0:128 = E_t (even q dims), t=0..127     -> psum bank qE
    rows 128:256 = O_t (odd  q dims), t=0..127     -> psum bank qO
    rows 256:384 = [E_t | O_t], t=128..191         -> psum bank qEO2
  Rope is elementwise on full-width [128, W] tiles (ACT casts psum->sbuf
  bf16 first so DVE runs in 2x mode); the E2/O2 mix uses duplicated trig
  tiles plus half-partition sub/add.
  KV^T per graph: dense 4-group [128x128] blocks, all 3 chunks in ONE psum
  bank; 32x32 diagonal blocks copied (4 strided DVE copies) into a single
  pre-zeroed block-diagonal arena [128, 3*128] whose column halves give the
  even/odd-row stationaries for 6 M_b matmuls.
  Out: per (col-chunk, window) 3-matmul accumulation q0/q1/q2 against
  mbE/mbO/mbEO2 stationaries; drains casted bf16 and DMA'd per window.
  PE stream is software-pipelined: ... KV(j), Mb(j-1), out(j-2) ... so
  cross-engine handoffs (DVE arena copies, ACT mb casts) are hidden.

Self-contained: hardcodes shapes; shards/pads on host inside kernel().
"""

import math
import os
from contextlib import ExitStack

import ml_dtypes
import numpy as np

import concourse.bacc as bacc_mod
import concourse.bass as bass  # noqa: F401
import concourse.mybir as mybir
import concourse.tile as tile
from concourse.bass_utils import run_bass_kernel_spmd


def _ensure_ntff_hook():
    """Register the axon NTFF profile hook if the image's antenv lacks it."""
    try:
        import antenv.axon_hooks  # noqa: F401

        return True
    except ImportError:
        pass
    try:
        import sys
        import types

        import antenv
        from trn_agent_boot.trn_boot import _ntff_profile_via_ctypes

        mod = types.ModuleType("antenv.axon_hooks")
        _hook = [None]
        mod.set_axon_ntff_profile_hook = lambda h: _hook.__setitem__(0, h)
        mod.get_axon_ntff_profile_hook = lambda: _hook[0]
        sys.modules["antenv.axon_hooks"] = mod
        antenv.axon_hooks = mod
        mod.set_axon_ntff_profile_hook(
            _ntff_profile_via_ctypes("/opt/axon/libaxon_pjrt.so")
        )
        return True
    except Exception:
        return False


FP32 = mybir.dt.float32
BF16 = mybir.dt.bfloat16
FP8 = mybir.dt.float8e4
AF = mybir.ActivationFunctionType

N = 32768
C = 384
E = 384
G = 12
D = 32
P = 16
SD = 3
NUM_GRAPHS = 64
NCORES = 8
GPD = NUM_GRAPHS // NCORES  # graphs per device
AVG = float(N) / NUM_GRAPHS  # 512.0
NT = 192  # rope pairs = G*P
W = 512  # streaming window


def _a_order_cols():
    """perm such that A-order column r is original q-dim perm[r].

    r in [0,128):   E_t, t=r        (q dim g*32 + 2p,   g=t//16, p=t%16)
    r in [128,256): O_t, t=r-128    (q dim g*32 + 2p+1)
    r in [256,320): E_t, t=128+(r-256)
    r in [320,384): O_t, t=128+(r-320)
    """
    perm = np.empty(E, dtype=np.int64)
    for r in range(E):
        if r < 128:
            t, odd = r, 0
        elif r < 256:
            t, odd = r - 128, 1
        elif r < 320:
            t, odd = 128 + (r - 256), 0
        else:
            t, odd = 128 + (r - 320), 1
        perm[r] = (t // 16) * 32 + 2 * (t % 16) + odd
    return perm


_APERM = _a_order_cols()

_CACHE = {}


def _build(slot: int, has_bias: bool):
    debug = bool(int(os.environ.get("PLATCONV_DEBUG", "0")))
    key = (slot, has_bias, debug)
    if key in _CACHE:
        return _CACHE[key]

    NP = GPD * slot
    NTILE = NP // 128
    TPS = slot // 128
    NCH = NP // W
    assert NP % W == 0
    HNP = NP // 2  # trig half split for earlier availability

    nc = bacc_mod.Bacc()

    CA = C + 1 if has_bias else C

    # x, pre-chunked/transposed on host for contiguous per-partition runs
    xt_d = nc.declare_dram_parameter("xt", [NCH, 128, 3, W], BF16, isOutput=False)
    if has_bias:
        xb_d = nc.declare_dram_parameter("xb", [1, NP], BF16, isOutput=False)
    # trig, feature-major: rows 0:128 = t<128; dup tiles hold t 128:192 twice
    cl_d = nc.declare_dram_parameter("cl", [128, NP], BF16, isOutput=False)
    sl_d = nc.declare_dram_parameter("sl", [128, NP], BF16, isOutput=False)
    cd_d = nc.declare_dram_parameter("cd", [128, NP], BF16, isOutput=False)
    sd_d = nc.declare_dram_parameter("sd", [128, NP], BF16, isOutput=False)
    # k, node-major pre-transposed: kn[p, t, e] = k[t*128+p, e]
    kn_d = nc.declare_dram_parameter("kn", [128, NP // 128, E], BF16, isOutput=False)
    wqa_d = nc.declare_dram_parameter("wqa", [CA, E], BF16, isOutput=False)
    wva_d = nc.declare_dram_parameter("wva", [CA, E], BF16, isOutput=False)
    wos_d = nc.declare_dram_parameter("wos", [E, C], BF16, isOutput=False)
    wosn_d = nc.declare_dram_parameter("wosn", [128, C], BF16, isOutput=False)
    out_d = nc.declare_dram_parameter("outt", [3, 128, NP], BF16, isOutput=True)
    if debug:
        dbg_q = nc.declare_dram_parameter("dbgq", [4, 128, NP], BF16, isOutput=True)
        dbg_v = nc.declare_dram_parameter("dbgv", [2, 128, NP // 128, E], BF16, isOutput=True)
        dbg_mb = nc.declare_dram_parameter("dbgmb", [GPD, 4, 128, C], BF16, isOutput=True)
        dbg_ar = nc.declare_dram_parameter("dbgar", [GPD, 128, 384], BF16, isOutput=True)

    with ExitStack() as ctx:
        tc = ctx.enter_context(tile.TileContext(nc))

        consts = ctx.enter_context(tc.tile_pool(name="consts", bufs=1))
        xtp = ctx.enter_context(tc.tile_pool(name="xtp", bufs=4))
        qsb = ctx.enter_context(tc.tile_pool(name="qsb", bufs=2))
        big = ctx.enter_context(tc.tile_pool(name="big", bufs=1))
        mbp = ctx.enter_context(tc.tile_pool(name="mbp", bufs=3))
        outp = ctx.enter_context(tc.tile_pool(name="outp", bufs=3))
        psum = ctx.enter_context(tc.tile_pool(name="psum", bufs=1, space="PSUM"))

        def pbank(tag):
            """One full PSUM bank ([128, 512] f32)."""
            return psum.tile([128, W], FP32, tag=tag, name=tag)

        # ---- weight loads; xt chunks 0/1 issued between wq and wv so the
        # first Q chain starts as early as possible ----
        wq_t = consts.tile([128, 3, E], BF16, tag="wq")
        nc.sync.dma_start(
            wq_t[:], wqa_d[0:C, :].rearrange("(b p) e -> p b e", p=128)
        )
        wv_t = consts.tile([128, 3, E], BF16, tag="wv")
        if has_bias:
            wqb = consts.tile([1, E], BF16, tag="wqb")
            nc.sync.dma_start(wqb[:], wqa_d[C : C + 1, :])
            wvb = consts.tile([1, E], BF16, tag="wvb")
            nc.sync.dma_start(wvb[:], wva_d[C : C + 1, :])

        # ---- persistent SBUF tensors ----
        q0 = big.tile([128, NP], BF16, tag="q0")  # E' rows t<128
        q1 = big.tile([128, NP], BF16, tag="q1")  # O' rows t<128
        # EO2 rope products, recombined on PE via mb2/mb2x stationaries:
        q2m1 = big.tile([128, NP], BF16, tag="q2m1")  # [E2*c2 ; O2*c2]
        q2m2 = big.tile([128, NP], BF16, tag="q2m2")  # [E2*s2 ; O2*s2]
        v_sb = big.tile([128, NTILE, E], BF16, tag="v_sb")
        k_sb = big.tile([128, NTILE, E], BF16, tag="k_sb")
        clf = big.tile([128, NP], BF16, tag="clf")
        slf = big.tile([128, NP], BF16, tag="slf")
        cdf = big.tile([128, NP], BF16, tag="cdf")  # [c2;c2] dup rows t>=128
        sdf = big.tile([128, NP], BF16, tag="sdf")  # [s2;s2]
        wos_t = consts.tile([128, 3, C], BF16, tag="wos")  # quad q rows
        wosn_t = consts.tile([128, C], BF16, tag="wosn")  # -wos, quad 2 rows

        # block-diag arena: cols [q*128 + eo*64 + m*16 + p]; zero once per set
        arenas = []
        for s in range(2):
            a = big.tile([128, 3 * 128], BF16, tag=f"arena{s}")
            nc.vector.memset(a[:], 0.0)
            arenas.append(a)

        # DMA issue schedule (sync queue is FIFO; order = priority).
        # xt prefetches are issued first in each iteration (see loop); the
        # extra loads below are staggered so each arrives before first use
        # without starving the xt stream.  EO2 rope muls for chunk ch are
        # emitted in iteration ch+1, so cdf/sdf may be emitted at ch==1.
        def extra_dmas(ch):
            if ch == 0:
                nc.sync.dma_start(clf[:, 0:HNP], cl_d[:, 0:HNP])
                nc.sync.dma_start(slf[:, 0:HNP], sl_d[:, 0:HNP])
            elif ch == 1:
                nc.sync.dma_start(cdf[:, 0:HNP], cd_d[:, 0:HNP])
                nc.sync.dma_start(sdf[:, 0:HNP], sd_d[:, 0:HNP])
            elif ch == 3:
                nc.sync.dma_start(clf[:, HNP:NP], cl_d[:, HNP:NP])
                nc.sync.dma_start(slf[:, HNP:NP], sl_d[:, HNP:NP])
            elif ch == 4:
                nc.sync.dma_start(cdf[:, HNP:NP], cd_d[:, HNP:NP])
                nc.sync.dma_start(sdf[:, HNP:NP], sd_d[:, HNP:NP])
            elif ch == 5:
                nc.sync.dma_start(
                    k_sb[:, 0 : NTILE // 2, :], kn_d[:, 0 : NTILE // 2, :]
                )
            elif ch == 6:
                nc.sync.dma_start(
                    k_sb[:, NTILE // 2 : NTILE, :], kn_d[:, NTILE // 2 : NTILE, :]
                )
            elif ch == 7:
                nc.sync.dma_start(
                    wos_t[:], wos_d[:].rearrange("(b p) e -> p b e", p=128)
                )
                nc.sync.dma_start(wosn_t[:], wosn_d[:])

        # ------------------------------------------------------------------
        # Phase A: Q/V projections + rope, per chunk of W nodes
        # psum tags: even chunk qE/qO/q2 = A0,A1,A2; odd = A3,A4,A5; v = A6,A7
        # ------------------------------------------------------------------
        def xt_load(ch):
            n0 = ch * W
            t = xtp.tile([128, 3, W], BF16, tag="xt")
            nc.sync.dma_start(t[:], xt_d[ch])
            if has_bias:
                tb = xtp.tile([1, W], BF16, tag="xtb")
                nc.sync.dma_start(tb[:], xb_d[:, n0 : n0 + W])
                return (t, tb)
            return (t, None)

        xt_tiles = {0: xt_load(0), 1: xt_load(1)}
        nc.sync.dma_start(
            wv_t[:], wva_d[0:C, :].rearrange("(b p) e -> p b e", p=128)
        )

        eo2_pending = []

        def emit_eo2(ch, q2s):
            n0 = ch * W
            nc.gpsimd.tensor_mul(
                q2m1[:, n0 : n0 + W], q2s[:], cdf[:, n0 : n0 + W]
            )
            nc.gpsimd.tensor_mul(
                q2m2[:, n0 : n0 + W], q2s[:], sdf[:, n0 : n0 + W]
            )

        for ch in range(NCH):
            n0 = ch * W
            par = ch % 2
            xt_c, xt_b = xt_tiles.pop(ch)
            if ch + 2 < NCH:
                xt_tiles[ch + 2] = xt_load(ch + 2)
            extra_dmas(ch)

            # Q banks interleaved with V subtile chains; ACT drains emitted
            # in readiness order so the ACT FIFO never head-of-line blocks.
            qEs = qsb.tile([128, W], BF16, tag="qEs")
            qOs = qsb.tile([128, W], BF16, tag="qOs")
            q2s = qsb.tile([128, W], BF16, tag="q2s")
            qcast = [qEs, qOs, q2s]
            qbank = [pbank(f"A{3 * par + i}") for i in range(3)]

            def q_chain(bk):
                c0 = 128 * bk
                for ki in range(3):
                    nc.tensor.matmul(
                        qbank[bk][:],
                        wq_t[:, ki, c0 : c0 + 128],
                        xt_c[:, ki, :],
                        start=(ki == 0),
                        stop=(ki == 2 and not has_bias),
                    )
                if has_bias:
                    nc.tensor.matmul(
                        qbank[bk][:],
                        wqb[:, c0 : c0 + 128],
                        xt_b[:],
                        start=False,
                        stop=True,
                    )
                nc.scalar.activation(qcast[bk][:], qbank[bk][:], AF.Copy)

            def v_chain(sub):
                ti = ch * (W // 128) + sub
                f0 = sub * 128
                vps = pbank(f"A{6 + sub % 2}")
                for ki in range(3):
                    nc.tensor.matmul(
                        vps[:, 0:E],
                        xt_c[:, ki, f0 : f0 + 128],
                        wv_t[:, ki, :],
                        start=(ki == 0),
                        stop=(ki == 2 and not has_bias),
                    )
                if has_bias:
                    nc.tensor.matmul(
                        vps[:, 0:E],
                        xt_b[:, f0 : f0 + 128],
                        wvb[:],
                        start=False,
                        stop=True,
                    )
                if sub == 3:
                    nc.vector.tensor_copy(v_sb[:, ti, :], vps[:, 0:E])
                else:
                    nc.scalar.activation(v_sb[:, ti, :], vps[:, 0:E], AF.Copy)

            q_chain(0)
            v_chain(0)
            q_chain(1)
            v_chain(1)
            q_chain(2)
            v_chain(2)
            v_chain(3)

            # rope (DVE, all [128, W], all operands partition-aligned)
            cw = lambda t: t[:, n0 : n0 + W]
            ta = qsb.tile([128, W], BF16, tag="ta")
            tb = qsb.tile([128, W], BF16, tag="tb")
            nc.vector.tensor_mul(ta[:], qEs[:], cw(clf))
            nc.vector.tensor_mul(tb[:], qOs[:], cw(slf))
            nc.vector.tensor_sub(cw(q0), ta[:], tb[:])
            nc.vector.tensor_mul(ta[:], qEs[:], cw(slf))
            nc.vector.tensor_mul(tb[:], qOs[:], cw(clf))
            nc.vector.tensor_add(cw(q1), ta[:], tb[:])
            # EO2 products on Pool (SBUF-only), deferred one iteration so
            # the cdf/sdf DMAs can be emitted after the xt stream starts;
            # PE recombines E2'/O2' via the mb2x stationary.
            for pch, pq2s in eo2_pending:
                emit_eo2(pch, pq2s)
            eo2_pending = [(ch, q2s)]

        # ------------------------------------------------------------------
        # Phase B per graph: KV (1 bank), arena copies, M_b (3 banks), out.
        # PE stream: KV(j), Mb(j-1), out(j-2) -- handoffs hidden.
        # psum tags: kvt = A0/A1 (j%2), mbE/mbO/mb2 = A2,A3,A4, out = A5/A6
        # ------------------------------------------------------------------
        mb_tiles = [None] * GPD  # sbuf stationaries per graph
        oti = 0

        def kv_stage(j):
            t0 = j * TPS
            kvt = pbank(f"A{j % 2}")
            for cchunk in range(3):
                for tt in range(TPS):
                    nc.tensor.matmul(
                        kvt[:, 128 * cchunk : 128 * (cchunk + 1)],
                        v_sb[:, t0 + tt, 128 * cchunk : 128 * (cchunk + 1)],
                        k_sb[:, t0 + tt, 128 * cchunk : 128 * (cchunk + 1)],
                        start=(tt == 0),
                        stop=(tt == TPS - 1),
                    )
            # arena copies: one strided DVE copy per m (3 groups each)
            # dst AP dims (q, p, eo): arena col = q*128 + eo*64 + 16*m + p
            A = arenas[j % 2]
            for m in range(4):
                r0 = 32 * m
                src = kvt[r0 : r0 + 32, 0 : 3 * 128].rearrange(
                    "e (q d) -> e q d", q=3
                )[:, :, r0 : r0 + 32].rearrange("e q (p eo) -> e q p eo", eo=2)
                dst = A[r0 : r0 + 32, :].rearrange(
                    "e (q eo hp p) -> e q p eo hp", q=3, eo=2, hp=4
                )[:, :, :, :, m]
                nc.vector.tensor_copy(dst, src)
            return kvt

        def mb_stage(j):
            A = arenas[j % 2]
            mbps = [pbank("A2"), pbank("A3"), pbank("A4")]
            # bank r: matmul quads with E-sel (r=0), O-sel (r=1), EO2 (r=2)
            for quad, (bank, colsel, mpos) in enumerate(
                (
                    (0, slice(0, 64), 0),
                    (0, slice(128, 192), 64),
                    (1, slice(64, 128), 0),
                    (1, slice(192, 256), 64),
                    (2, slice(256, 320), 0),
                    (2, slice(320, 384), 64),
                )
            ):
                q = colsel.start // 128
                nc.tensor.matmul(
                    mbps[bank][mpos : mpos + 64, 0:C],
                    A[:, colsel],
                    wos_t[:, q, :],
                    start=True,
                    stop=True,
                    tile_position=(0, mpos),
                )
            # mb2x = [M[O2-rows] ; -M[E2-rows]] for the q2m2 accumulation term
            mb2xps = pbank("A7")
            nc.tensor.matmul(
                mb2xps[0:64, 0:C],
                A[:, 320:384],
                wos_t[:, 2, :],
                start=True,
                stop=True,
                tile_position=(0, 0),
            )
            nc.tensor.matmul(
                mb2xps[64:128, 0:C],
                A[:, 256:320],
                wosn_t[:],
                start=True,
                stop=True,
                tile_position=(0, 64),
            )
            # casts to sbuf stationaries: 2 ACT + 2 DVE
            mb0 = mbp.tile([128, C], BF16, tag="mb0")
            mb1 = mbp.tile([128, C], BF16, tag="mb1")
            mb2 = mbp.tile([128, C], BF16, tag="mb2")
            mb2x = mbp.tile([128, C], BF16, tag="mb2x")
            nc.scalar.activation(mb0[:], mbps[0][:, 0:C], AF.Copy)
            nc.scalar.activation(mb1[:], mbps[1][:, 0:C], AF.Copy)
            nc.vector.tensor_copy(mb2[:], mbps[2][:, 0:C])
            nc.vector.tensor_copy(mb2x[:], mb2xps[:, 0:C])
            mb_tiles[j] = (mb0, mb1, mb2, mb2x)
            if debug:
                for i, t in enumerate((mb0, mb1, mb2, mb2x)):
                    nc.sync.dma_start(dbg_mb[j, i], t[:])
                nc.sync.dma_start(dbg_ar[j], A[:])

        def out_stage(j):
            nonlocal oti
            mb0, mb1, mb2, mb2x = mb_tiles[j]
            slot0 = j * slot
            wins = []
            o = 0
            while o < slot:
                w = min(W, slot - o)
                wins.append((slot0 + o, w))
                o += w
            for w0, w in wins:
                ost = outp.tile([128, 3, W], BF16, tag="ost")
                for cch in range(3):
                    cc = slice(128 * cch, 128 * (cch + 1))
                    ot = pbank(f"A{5 + oti % 2}")
                    nc.tensor.matmul(
                        ot[:, :w], mb0[:, cc], q0[:, w0 : w0 + w],
                        start=True, stop=False,
                    )
                    nc.tensor.matmul(
                        ot[:, :w], mb1[:, cc], q1[:, w0 : w0 + w],
                        start=False, stop=False,
                    )
                    nc.tensor.matmul(
                        ot[:, :w], mb2[:, cc], q2m1[:, w0 : w0 + w],
                        start=False, stop=False,
                    )
                    nc.tensor.matmul(
                        ot[:, :w], mb2x[:, cc], q2m2[:, w0 : w0 + w],
                        start=False, stop=True,
                    )
                    # drains: alternate ACT/DVE (Pool cannot read PSUM)
                    if oti % 2 == 0:
                        nc.scalar.activation(ost[:, cch, :w], ot[:, :w], AF.Copy)
                    else:
                        nc.vector.tensor_copy(ost[:, cch, :w], ot[:, :w])
                    oti += 1
                    nc.sync.dma_start(
                        out_d[cch, :, w0 : w0 + w], ost[:, cch, :w]
                    )

        for pch, pq2s in eo2_pending:
            emit_eo2(pch, pq2s)
        eo2_pending = []

        # software pipeline (out lags by 3 so mb casts are never waited on)
        kv_stage(0)
        kv_stage(1)
        mb_stage(0)
        kv_stage(2)
        mb_stage(1)
        for j in range(3, GPD):
            kv_stage(j)
            mb_stage(j - 1)
            out_stage(j - 3)
        mb_stage(GPD - 1)
        out_stage(GPD - 3)
        out_stage(GPD - 2)
        out_stage(GPD - 1)

        if debug:
            for i, t in enumerate((q0, q1, q2m1, q2m2)):
                nc.sync.dma_start(dbg_q[i], t[:])
            nc.sync.dma_start(dbg_v[0], v_sb[:])
            nc.sync.dma_start(dbg_v[1], k_sb[:])

    nc.compile()

    _CACHE[key] = (nc, NP)
    return nc, NP


last_exec_time_ns = None
last_results = None


def kernel(x, pos, batch, Wq, bq, Wv, bv, Wo, bo, freqs):
    global last_exec_time_ns
    x = np.asarray(x, dtype=np.float32)
    pos = np.asarray(pos, dtype=np.float32)
    batch = np.asarray(batch).astype(np.int64)
    Wq = np.asarray(Wq, dtype=np.float32)
    bq = np.asarray(bq, dtype=np.float32)
    Wv = np.asarray(Wv, dtype=np.float32)
    bv = np.asarray(bv, dtype=np.float32)
    Wo = np.asarray(Wo, dtype=np.float32)
    bo = np.asarray(bo, dtype=np.float32)
    freqs = np.asarray(freqs, dtype=np.float32)

    counts = np.bincount(batch, minlength=NUM_GRAPHS)
    starts = np.concatenate([[0], np.cumsum(counts)])
    slot = max(640, int(math.ceil(counts.max() / 128.0)) * 128)
    has_bias = bool(np.any(bq) or np.any(bv))

    nc, NP = _build(slot, has_bias)

    WqA = Wq[:, _APERM]
    bqA = bq[_APERM]
    CA = C + 1 if has_bias else C
    bf = ml_dtypes.bfloat16

    wqa = np.zeros((CA, E), dtype=bf)
    wqa[:C] = WqA.astype(bf)
    wva = np.zeros((CA, E), dtype=bf)
    wva[:C] = Wv.astype(bf)
    if has_bias:
        wqa[C] = bqA.astype(bf)
        wva[C] = bv.astype(bf)
    wosf = Wo * (1.0 / AVG)
    wos = wosf.astype(bf)
    wosn = (-wosf[256:384]).astype(bf)

    # phase & trig on host (t = g*16+p, g-major)
    fr = freqs.reshape(NT, SD)
    phase = pos @ fr.T  # [N, 192] float32
    cphase = np.cos(phase)
    sphase = np.sin(phase)
    # k node-major: col g*32+2p = c - s (even), col g*32+2p+1 = c + s (odd)
    kfull = np.empty((len(x), E), dtype=np.float32)
    k3 = kfull.reshape(len(x), G, P, 2)
    ph3c = cphase.reshape(len(x), G, P)
    ph3s = sphase.reshape(len(x), G, P)
    k3[:, :, :, 0] = ph3c - ph3s
    k3[:, :, :, 1] = ph3c + ph3s

    NCH = NP // W
    NTILE = NP // 128
    in_maps = []
    for d in range(NCORES):
        xtf = np.zeros((C, NP), dtype=np.float32)
        xb = np.zeros((1, NP), dtype=bf)
        cl = np.zeros((128, NP), dtype=bf)
        sl = np.zeros((128, NP), dtype=bf)
        cd = np.zeros((128, NP), dtype=bf)
        sd = np.zeros((128, NP), dtype=bf)
        knf = np.zeros((NP, E), dtype=np.float32)
        for lj in range(GPD):
            gb = d * GPD + lj
            s, e_, cnt = starts[gb], starts[gb + 1], counts[gb]
            if cnt == 0:
                continue
            o = lj * slot
            xtf[:, o : o + cnt] = x[s:e_].T
            if has_bias:
                xb[0, o : o + cnt] = 1.0
            cl[:, o : o + cnt] = cphase[s:e_, 0:128].T.astype(bf)
            sl[:, o : o + cnt] = sphase[s:e_, 0:128].T.astype(bf)
            cd[0:64, o : o + cnt] = cphase[s:e_, 128:NT].T.astype(bf)
            cd[64:128, o : o + cnt] = cphase[s:e_, 128:NT].T.astype(bf)
            sd[0:64, o : o + cnt] = sphase[s:e_, 128:NT].T.astype(bf)
            sd[64:128, o : o + cnt] = sphase[s:e_, 128:NT].T.astype(bf)
            knf[o : o + cnt, :] = kfull[s:e_]
        # pre-chunk/transpose for contiguous per-partition DMA runs
        xt = np.ascontiguousarray(
            xtf.reshape(3, 128, NCH, W).transpose(2, 1, 0, 3)
        ).astype(bf)
        kn = np.ascontiguousarray(
            knf.reshape(NTILE, 128, E).transpose(1, 0, 2)
        ).astype(bf)
        m = {
            "xt": xt,
            "cl": cl,
            "sl": sl,
            "cd": cd,
            "sd": sd,
            "kn": kn,
            "wqa": wqa,
            "wva": wva,
            "wos": wos,
            "wosn": wosn,
        }
        if has_bias:
            m["xb"] = xb
        in_maps.append(m)

    want_trace = bool(int(os.environ.get("PLATCONV_TRACE", "0")))
    if want_trace:
        want_trace = _ensure_ntff_hook()
    res = run_bass_kernel_spmd(
        nc,
        in_maps,
        core_ids=list(range(NCORES)),
        trace=want_trace,
    )
    last_exec_time_ns = res.exec_time_ns
    global last_results
    last_results = res

    out = np.zeros((N, C), dtype=np.float32)
    for d in range(NCORES):
        ot = np.asarray(res.results[d]["outt"], dtype=np.float32).reshape(C, NP)
        for lj in range(GPD):
            gb = d * GPD + lj
            s, e_, cnt = starts[gb], starts[gb + 1], counts[gb]
            if cnt == 0:
                continue
            o = lj * slot
            out[s:e_] = ot[:, o : o + cnt].T
    out += bo[None, :]
    return out


# revision 43
# speedup vs baseline: 1.0952x; 1.0952x over previous
"""Trainium2 Bass kernel for nn_PlatonicConv (linear-attention GNN message passing).

Math (reference):
  q = rope(x@Wq + bq, phase);  k = rope(ones, phase);  v = x@Wv + bv
  phase[n, g, p] = pos[n, :] . freqs[g, 0, p, :]
  KV_b[g] = (1/AVG) * sum_{n in graph b} k[n,g,:] (x) v[n,g,:]
  out[n]  = concat_g( q'[n,g,:] @ KV_b[g] ) @ Wo + bo

Device formulation (per core, data-parallel over graphs; 8 graphs/core):
  host precomputes cos/sin of phase (feature-major, bf16) and node-major
  k = rope(ones) (unscaled; 1/AVG folded into Wo).  Per graph b:
    M_b = stack_rows(KV_b[g] @ Wo[g-rows]) : [384, 384]
    out[n] = q'[n] @ M_{b(n)}  (+ bo on host).
  q'/M_b rows use "A-order" over rope pairs t = g*16+p:
    rows   0:128 = E_t (even q dims), t=0..127     -> psum bank qE
    rows 128:256 = O_t (odd  q dims), t=0..127     -> psum bank qO
    rows 256:384 = [E_t | O_t], t=128..191         -> psum bank qEO2
  Rope is elementwise on full-width [128, W] tiles (ACT casts psum->sbuf
  bf16 first so DVE runs in 2x mode); the E2/O2 mix uses duplicated trig
  tiles plus half-partition sub/add.
  KV^T per graph: dense 4-group [128x128] blocks, all 3 chunks in ONE psum
  bank; 32x32 diagonal blocks copied (4 strided DVE copies) into a single
  pre-zeroed block-diagonal arena [128, 3*128] whose column halves give the
  even/odd-row stationaries for 6 M_b matmuls.
  Out: per (col-chunk, window) 3-matmul accumulation q0/q1/q2 against
  mbE/mbO/mbEO2 stationaries; drains casted bf16 and DMA'd per window.
  PE stream is software-pipelined: ... KV(j), Mb(j-1), out(j-2) ... so
  cross-engine handoffs (DVE arena copies, ACT mb casts) are hidden.

Self-contained: hardcodes shapes; shards/pads on host inside kernel().
"""

import math
import os
from contextlib import ExitStack

import ml_dtypes
import numpy as np

import concourse.bacc as bacc_mod
import concourse.bass as bass  # noqa: F401
import concourse.mybir as mybir
import concourse.tile as tile
from concourse.bass_utils import run_bass_kernel_spmd


def _ensure_ntff_hook():
    """Register the axon NTFF profile hook if the image's antenv lacks it."""
    try:
        import antenv.axon_hooks  # noqa: F401

        return True
    except ImportError:
        pass
    try:
        import sys
        import types

        import antenv
        from trn_agent_boot.trn_boot import _ntff_profile_via_ctypes

        mod = types.ModuleType("antenv.axon_hooks")
        _hook = [None]
        mod.set_axon_ntff_profile_hook = lambda h: _hook.__setitem__(0, h)
        mod.get_axon_ntff_profile_hook = lambda: _hook[0]
        sys.modules["antenv.axon_hooks"] = mod
        antenv.axon_hooks = mod
        mod.set_axon_ntff_profile_hook(
            _ntff_profile_via_ctypes("/opt/axon/libaxon_pjrt.so")
        )
        return True
    except Exception:
        return False


FP32 = mybir.dt.float32
BF16 = mybir.dt.bfloat16
FP8 = mybir.dt.float8e4
AF = mybir.ActivationFunctionType

N = 32768
C = 384
E = 384
G = 12
D = 32
P = 16
SD = 3
NUM_GRAPHS = 64
NCORES = 8
GPD = NUM_GRAPHS // NCORES  # graphs per device
AVG = float(N) / NUM_GRAPHS  # 512.0
NT = 192  # rope pairs = G*P
W = 512  # streaming window


def _a_order_cols():
    """perm such that A-order column r is original q-dim perm[r].

    r in [0,128):   E_t, t=r        (q dim g*32 + 2p,   g=t//16, p=t%16)
    r in [128,256): O_t, t=r-128    (q dim g*32 + 2p+1)
    r in [256,320): E_t, t=128+(r-256)
    r in [320,384): O_t, t=128+(r-320)
    """
    perm = np.empty(E, dtype=np.int64)
    for r in range(E):
        if r < 128:
            t, odd = r, 0
        elif r < 256:
            t, odd = r - 128, 1
        elif r < 320:
            t, odd = 128 + (r - 256), 0
        else:
            t, odd = 128 + (r - 320), 1
        perm[r] = (t // 16) * 32 + 2 * (t % 16) + odd
    return perm


_APERM = _a_order_cols()

_CACHE = {}


def _build(slot: int, has_bias: bool):
    debug = bool(int(os.environ.get("PLATCONV_DEBUG", "0")))
    key = (slot, has_bias, debug)
    if key in _CACHE:
        return _CACHE[key]

    NP = GPD * slot
    NTILE = NP // 128
    TPS = slot // 128
    NCH = NP // W
    assert NP % W == 0
    HNP = NP // 2  # trig half split for earlier availability

    nc = bacc_mod.Bacc()

    CA = C + 1 if has_bias else C

    # x, pre-chunked/transposed on host for contiguous per-partition runs
    xt_d = nc.declare_dram_parameter("xt", [NCH, 128, 3, W], BF16, isOutput=False)
    if has_bias:
        xb_d = nc.declare_dram_parameter("xb", [1, NP], BF16, isOutput=False)
    # trig, feature-major: rows 0:128 = t<128; dup tiles hold t 128:192 twice
    cl_d = nc.declare_dram_parameter("cl", [128, NP], BF16, isOutput=False)
    sl_d = nc.declare_dram_parameter("sl", [128, NP], BF16, isOutput=False)
    cd_d = nc.declare_dram_parameter("cd", [128, NP], BF16, isOutput=False)
    sd_d = nc.declare_dram_parameter("sd", [128, NP], BF16, isOutput=False)
    # k, node-major pre-transposed: kn[p, t, e] = k[t*128+p, e]
    kn_d = nc.declare_dram_parameter("kn", [128, NP // 128, E], BF16, isOutput=False)
    wqa_d = nc.declare_dram_parameter("wqa", [CA, E], BF16, isOutput=False)
    wva_d = nc.declare_dram_parameter("wva", [CA, E], BF16, isOutput=False)
    wos_d = nc.declare_dram_parameter("wos", [E, C], BF16, isOutput=False)
    wosn_d = nc.declare_dram_parameter("wosn", [128, C], BF16, isOutput=False)
    out_d = nc.declare_dram_parameter("outt", [3, 128, NP], BF16, isOutput=True)
    if debug:
        dbg_q = nc.declare_dram_parameter("dbgq", [4, 128, NP], BF16, isOutput=True)
        dbg_v = nc.declare_dram_parameter("dbgv", [2, 128, NP // 128, E], BF16, isOutput=True)
        dbg_mb = nc.declare_dram_parameter("dbgmb", [GPD, 4, 128, C], BF16, isOutput=True)
        dbg_ar = nc.declare_dram_parameter("dbgar", [GPD, 128, 384], BF16, isOutput=True)

    with ExitStack() as ctx:
        tc = ctx.enter_context(tile.TileContext(nc))

        consts = ctx.enter_context(tc.tile_pool(name="consts", bufs=1))
        xtp = ctx.enter_context(tc.tile_pool(name="xtp", bufs=4))
        qsb = ctx.enter_context(tc.tile_pool(name="qsb", bufs=2))
        big = ctx.enter_context(tc.tile_pool(name="big", bufs=1))
        mbp = ctx.enter_context(tc.tile_pool(name="mbp", bufs=3))
        outp = ctx.enter_context(tc.tile_pool(name="outp", bufs=3))
        psum = ctx.enter_context(tc.tile_pool(name="psum", bufs=1, space="PSUM"))

        def pbank(tag):
            """One full PSUM bank ([128, 512] f32)."""
            return psum.tile([128, W], FP32, tag=tag, name=tag)

        # ---- weight loads; xt chunks 0/1 issued between wq and wv so the
        # first Q chain starts as early as possible ----
        wq_t = consts.tile([128, 3, E], BF16, tag="wq")
        nc.sync.dma_start(
            wq_t[:], wqa_d[0:C, :].rearrange("(b p) e -> p b e", p=128)
        )
        wv_t = consts.tile([128, 3, E], BF16, tag="wv")
        if has_bias:
            wqb = consts.tile([1, E], BF16, tag="wqb")
            nc.sync.dma_start(wqb[:], wqa_d[C : C + 1, :])
            wvb = consts.tile([1, E], BF16, tag="wvb")
            nc.sync.dma_start(wvb[:], wva_d[C : C + 1, :])

        # ---- persistent SBUF tensors ----
        q0 = big.tile([128, NP], BF16, tag="q0")  # E' rows t<128
        q1 = big.tile([128, NP], BF16, tag="q1")  # O' rows t<128
        # EO2 rope products, recombined on PE via mb2/mb2x stationaries:
        q2m1 = big.tile([128, NP], BF16, tag="q2m1")  # [E2*c2 ; O2*c2]
        q2m2 = big.tile([128, NP], BF16, tag="q2m2")  # [E2*s2 ; O2*s2]
        v_sb = big.tile([128, NTILE, E], BF16, tag="v_sb")
        k_sb = big.tile([128, NTILE, E], BF16, tag="k_sb")
        clf = big.tile([128, NP], BF16, tag="clf")
        slf = big.tile([128, NP], BF16, tag="slf")
        cdf = big.tile([128, NP], BF16, tag="cdf")  # [c2;c2] dup rows t>=128
        sdf = big.tile([128, NP], BF16, tag="sdf")  # [s2;s2]
        wos_t = consts.tile([128, 3, C], BF16, tag="wos")  # quad q rows
        wosn_t = consts.tile([128, C], BF16, tag="wosn")  # -wos, quad 2 rows

        # block-diag arena: cols [q*128 + eo*64 + m*16 + p]; zero once per set
        arenas = []
        for s in range(2):
            a = big.tile([128, 3 * 128], BF16, tag=f"arena{s}")
            nc.vector.memset(a[:], 0.0)
            arenas.append(a)

        # DMA issue schedule (sync queue is FIFO; order = priority).
        # xt prefetches are issued first in each iteration (see loop); the
        # extra loads below are staggered so each arrives before first use
        # without starving the xt stream.  EO2 rope muls for chunk ch are
        # emitted in iteration ch+1, so cdf/sdf may be emitted at ch==1.
        def extra_dmas(ch):
            if ch == 0:
                nc.sync.dma_start(clf[:, 0:HNP], cl_d[:, 0:HNP])
                nc.sync.dma_start(slf[:, 0:HNP], sl_d[:, 0:HNP])
            elif ch == 1:
                nc.sync.dma_start(cdf[:, 0:HNP], cd_d[:, 0:HNP])
                nc.sync.dma_start(sdf[:, 0:HNP], sd_d[:, 0:HNP])
            elif ch == 3:
                nc.sync.dma_start(clf[:, HNP:NP], cl_d[:, HNP:NP])
                nc.sync.dma_start(slf[:, HNP:NP], sl_d[:, HNP:NP])
            elif ch == 4:
                nc.sync.dma_start(cdf[:, HNP:NP], cd_d[:, HNP:NP])
                nc.sync.dma_start(sdf[:, HNP:NP], sd_d[:, HNP:NP])
            elif ch == 6:
                nc.sync.dma_start(
                    k_sb[:, 0 : NTILE // 2, :], kn_d[:, 0 : NTILE // 2, :]
                )
            elif ch == 7:
                nc.sync.dma_start(
                    k_sb[:, NTILE // 2 : NTILE, :], kn_d[:, NTILE // 2 : NTILE, :]
                )
            elif ch == 8:
                nc.sync.dma_start(
                    wos_t[:], wos_d[:].rearrange("(b p) e -> p b e", p=128)
                )
                nc.sync.dma_start(wosn_t[:], wosn_d[:])

        # ------------------------------------------------------------------
        # Phase A: Q/V projections + rope, per chunk of W nodes
        # psum tags: even chunk qE/qO/q2 = A0,A1,A2; odd = A3,A4,A5; v = A6,A7
        # ------------------------------------------------------------------
        def xt_load(ch):
            n0 = ch * W
            t = xtp.tile([128, 3, W], BF16, tag="xt")
            nc.sync.dma_start(t[:], xt_d[ch])
            if has_bias:
                tb = xtp.tile([1, W], BF16, tag="xtb")
                nc.sync.dma_start(tb[:], xb_d[:, n0 : n0 + W])
                return (t, tb)
            return (t, None)

        xt_tiles = {0: xt_load(0), 1: xt_load(1)}
        nc.sync.dma_start(
            wv_t[:], wva_d[0:C, :].rearrange("(b p) e -> p b e", p=128)
        )

        eo2_pending = []

        def emit_eo2(ch, q2s):
            n0 = ch * W
            nc.gpsimd.tensor_mul(
                q2m1[:, n0 : n0 + W], q2s[:], cdf[:, n0 : n0 + W]
            )
            nc.gpsimd.tensor_mul(
                q2m2[:, n0 : n0 + W], q2s[:], sdf[:, n0 : n0 + W]
            )

        for ch in range(NCH):
            n0 = ch * W
            par = ch % 2
            xt_c, xt_b = xt_tiles.pop(ch)
            if ch + 2 < NCH:
                xt_tiles[ch + 2] = xt_load(ch + 2)
            extra_dmas(ch)

            # Q banks interleaved with V subtile chains; ACT drains emitted
            # in readiness order so the ACT FIFO never head-of-line blocks.
            qEs = qsb.tile([128, W], BF16, tag="qEs")
            qOs = qsb.tile([128, W], BF16, tag="qOs")
            q2s = qsb.tile([128, W], BF16, tag="q2s")
            qcast = [qEs, qOs, q2s]
            qbank = [pbank(f"A{3 * par + i}") for i in range(3)]

            def q_chain(bk):
                c0 = 128 * bk
                for ki in range(3):
                    nc.tensor.matmul(
                        qbank[bk][:],
                        wq_t[:, ki, c0 : c0 + 128],
                        xt_c[:, ki, :],
                        start=(ki == 0),
                        stop=(ki == 2 and not has_bias),
                    )
                if has_bias:
                    nc.tensor.matmul(
                        qbank[bk][:],
                        wqb[:, c0 : c0 + 128],
                        xt_b[:],
                        start=False,
                        stop=True,
                    )
                nc.scalar.activation(qcast[bk][:], qbank[bk][:], AF.Copy)

            def v_chain(sub):
                ti = ch * (W // 128) + sub
                f0 = sub * 128
                vps = pbank(f"A{6 + sub % 2}")
                for ki in range(3):
                    nc.tensor.matmul(
                        vps[:, 0:E],
                        xt_c[:, ki, f0 : f0 + 128],
                        wv_t[:, ki, :],
                        start=(ki == 0),
                        stop=(ki == 2 and not has_bias),
                    )
                if has_bias:
                    nc.tensor.matmul(
                        vps[:, 0:E],
                        xt_b[:, f0 : f0 + 128],
                        wvb[:],
                        start=False,
                        stop=True,
                    )
                if sub == 3:
                    nc.vector.tensor_copy(v_sb[:, ti, :], vps[:, 0:E])
                else:
                    nc.scalar.activation(v_sb[:, ti, :], vps[:, 0:E], AF.Copy)

            q_chain(0)
            v_chain(0)
            q_chain(1)
            v_chain(1)
            q_chain(2)
            v_chain(2)
            v_chain(3)

            # rope (DVE, all [128, W], all operands partition-aligned)
            cw = lambda t: t[:, n0 : n0 + W]
            ta = qsb.tile([128, W], BF16, tag="ta")
            tb = qsb.tile([128, W], BF16, tag="tb")
            nc.vector.tensor_mul(ta[:], qEs[:], cw(clf))
            nc.vector.tensor_mul(tb[:], qOs[:], cw(slf))
            nc.vector.tensor_sub(cw(q0), ta[:], tb[:])
            nc.vector.tensor_mul(ta[:], qEs[:], cw(slf))
            nc.vector.tensor_mul(tb[:], qOs[:], cw(clf))
            nc.vector.tensor_add(cw(q1), ta[:], tb[:])
            # EO2 products on Pool (SBUF-only), deferred one iteration so
            # the cdf/sdf DMAs can be emitted after the xt stream starts;
            # PE recombines E2'/O2' via the mb2x stationary.
            for pch, pq2s in eo2_pending:
                emit_eo2(pch, pq2s)
            eo2_pending = [(ch, q2s)]

        # ------------------------------------------------------------------
        # Phase B per graph: KV (1 bank), arena copies, M_b (3 banks), out.
        # PE stream: KV(j), Mb(j-1), out(j-2) -- handoffs hidden.
        # psum tags: kvt = A0/A1 (j%2), mbE/mbO/mb2 = A2,A3,A4, out = A5/A6
        # ------------------------------------------------------------------
        mb_tiles = [None] * GPD  # sbuf stationaries per graph
        oti = 0

        def kv_stage(j):
            t0 = j * TPS
            kvt = pbank("A0")
            for cchunk in range(3):
                for tt in range(TPS):
                    nc.tensor.matmul(
                        kvt[:, 128 * cchunk : 128 * (cchunk + 1)],
                        v_sb[:, t0 + tt, 128 * cchunk : 128 * (cchunk + 1)],
                        k_sb[:, t0 + tt, 128 * cchunk : 128 * (cchunk + 1)],
                        start=(tt == 0),
                        stop=(tt == TPS - 1),
                    )
            # arena copies: one DVE copy per m (3 groups each); k columns are
            # host-ordered d' = eo*16 + p, so both src and dst iterate
            # (q, eo, p) with contiguous 16-element p-runs.
            A = arenas[j % 2]
            for m in range(4):
                r0 = 32 * m
                src = kvt[r0 : r0 + 32, 0 : 3 * 128].rearrange(
                    "e (q d) -> e q d", q=3
                )[:, :, r0 : r0 + 32].rearrange("e q (eo p) -> e q eo p", eo=2)
                dst = A[r0 : r0 + 32, :].rearrange(
                    "e (q eo hp p) -> e q eo p hp", q=3, eo=2, hp=4
                )[:, :, :, :, m]
                nc.vector.tensor_copy(dst, src)
            return kvt

        def mb_stage(j):
            A = arenas[j % 2]
            mbps = [pbank("A2"), pbank("A3"), pbank("A4")]
            # bank r: matmul quads with E-sel (r=0), O-sel (r=1), EO2 (r=2)
            for quad, (bank, colsel, mpos) in enumerate(
                (
                    (0, slice(0, 64), 0),
                    (0, slice(128, 192), 64),
                    (1, slice(64, 128), 0),
                    (1, slice(192, 256), 64),
                    (2, slice(256, 320), 0),
                    (2, slice(320, 384), 64),
                )
            ):
                q = colsel.start // 128
                nc.tensor.matmul(
                    mbps[bank][mpos : mpos + 64, 0:C],
                    A[:, colsel],
                    wos_t[:, q, :],
                    start=True,
                    stop=True,
                    tile_position=(0, mpos),
                )
            # mb2x = [M[O2-rows] ; -M[E2-rows]] for the q2m2 accumulation term
            mb2xps = pbank("A7")
            nc.tensor.matmul(
                mb2xps[0:64, 0:C],
                A[:, 320:384],
                wos_t[:, 2, :],
                start=True,
                stop=True,
                tile_position=(0, 0),
            )
            nc.tensor.matmul(
                mb2xps[64:128, 0:C],
                A[:, 256:320],
                wosn_t[:],
                start=True,
                stop=True,
                tile_position=(0, 64),
            )
            # casts to sbuf stationaries: 2 ACT + 2 DVE
            mb0 = mbp.tile([128, C], BF16, tag="mb0")
            mb1 = mbp.tile([128, C], BF16, tag="mb1")
            mb2 = mbp.tile([128, C], BF16, tag="mb2")
            mb2x = mbp.tile([128, C], BF16, tag="mb2x")
            nc.scalar.activation(mb0[:], mbps[0][:, 0:C], AF.Copy)
            nc.scalar.activation(mb1[:], mbps[1][:, 0:C], AF.Copy)
            nc.vector.tensor_copy(mb2[:], mbps[2][:, 0:C])
            nc.vector.tensor_copy(mb2x[:], mb2xps[:, 0:C])
            mb_tiles[j] = (mb0, mb1, mb2, mb2x)
            if debug:
                for i, t in enumerate((mb0, mb1, mb2, mb2x)):
                    nc.sync.dma_start(dbg_mb[j, i], t[:])
                nc.sync.dma_start(dbg_ar[j], A[:])

        def out_stage(j):
            nonlocal oti
            mb0, mb1, mb2, mb2x = mb_tiles[j]
            slot0 = j * slot
            wins = []
            o = 0
            while o < slot:
                w = min(W, slot - o)
                wins.append((slot0 + o, w))
                o += w
            for w0, w in wins:
                ost = outp.tile([128, 3, W], BF16, tag="ost")
                for cch in range(3):
                    cc = slice(128 * cch, 128 * (cch + 1))
                    ot = pbank(("A5", "A6", "A1")[oti % 3])
                    nc.tensor.matmul(
                        ot[:, :w], mb0[:, cc], q0[:, w0 : w0 + w],
                        start=True, stop=False,
                    )
                    nc.tensor.matmul(
                        ot[:, :w], mb1[:, cc], q1[:, w0 : w0 + w],
                        start=False, stop=False,
                    )
                    nc.tensor.matmul(
                        ot[:, :w], mb2[:, cc], q2m1[:, w0 : w0 + w],
                        start=False, stop=False,
                    )
                    nc.tensor.matmul(
                        ot[:, :w], mb2x[:, cc], q2m2[:, w0 : w0 + w],
                        start=False, stop=True,
                    )
                    # drains: alternate ACT/DVE (Pool cannot read PSUM)
                    if oti % 2 == 0:
                        nc.scalar.activation(ost[:, cch, :w], ot[:, :w], AF.Copy)
                    else:
                        nc.vector.tensor_copy(ost[:, cch, :w], ot[:, :w])
                    oti += 1
                    nc.sync.dma_start(
                        out_d[cch, :, w0 : w0 + w], ost[:, cch, :w]
                    )

        for pch, pq2s in eo2_pending:
            emit_eo2(pch, pq2s)
        eo2_pending = []

        # software pipeline (out lags by 3 so mb casts are never waited on)
        kv_stage(0)
        kv_stage(1)
        mb_stage(0)
        kv_stage(2)
        mb_stage(1)
        for j in range(3, GPD):
            kv_stage(j)
            mb_stage(j - 1)
            out_stage(j - 3)
        mb_stage(GPD - 1)
        out_stage(GPD - 3)
        out_stage(GPD - 2)
        out_stage(GPD - 1)

        if debug:
            for i, t in enumerate((q0, q1, q2m1, q2m2)):
                nc.sync.dma_start(dbg_q[i], t[:])
            nc.sync.dma_start(dbg_v[0], v_sb[:])
            nc.sync.dma_start(dbg_v[1], k_sb[:])

    nc.compile()

    _CACHE[key] = (nc, NP)
    return nc, NP


last_exec_time_ns = None
last_results = None


def kernel(x, pos, batch, Wq, bq, Wv, bv, Wo, bo, freqs):
    global last_exec_time_ns
    x = np.asarray(x, dtype=np.float32)
    pos = np.asarray(pos, dtype=np.float32)
    batch = np.asarray(batch).astype(np.int64)
    Wq = np.asarray(Wq, dtype=np.float32)
    bq = np.asarray(bq, dtype=np.float32)
    Wv = np.asarray(Wv, dtype=np.float32)
    bv = np.asarray(bv, dtype=np.float32)
    Wo = np.asarray(Wo, dtype=np.float32)
    bo = np.asarray(bo, dtype=np.float32)
    freqs = np.asarray(freqs, dtype=np.float32)

    counts = np.bincount(batch, minlength=NUM_GRAPHS)
    starts = np.concatenate([[0], np.cumsum(counts)])
    slot = max(640, int(math.ceil(counts.max() / 128.0)) * 128)
    has_bias = bool(np.any(bq) or np.any(bv))

    nc, NP = _build(slot, has_bias)

    WqA = Wq[:, _APERM]
    bqA = bq[_APERM]
    CA = C + 1 if has_bias else C
    bf = ml_dtypes.bfloat16

    wqa = np.zeros((CA, E), dtype=bf)
    wqa[:C] = WqA.astype(bf)
    wva = np.zeros((CA, E), dtype=bf)
    wva[:C] = Wv.astype(bf)
    if has_bias:
        wqa[C] = bqA.astype(bf)
        wva[C] = bv.astype(bf)
    wosf = Wo * (1.0 / AVG)
    wos = wosf.astype(bf)
    wosn = (-wosf[256:384]).astype(bf)

    # phase & trig on host (t = g*16+p, g-major)
    fr = freqs.reshape(NT, SD)
    phase = pos @ fr.T  # [N, 192] float32
    cphase = np.cos(phase)
    sphase = np.sin(phase)
    # k node-major, eo-grouped cols: col g*32 + eo*16 + p, eo=0: c-s, eo=1: c+s
    kfull = np.empty((len(x), E), dtype=np.float32)
    k3 = kfull.reshape(len(x), G, 2, P)
    ph3c = cphase.reshape(len(x), G, P)
    ph3s = sphase.reshape(len(x), G, P)
    k3[:, :, 0, :] = ph3c - ph3s
    k3[:, :, 1, :] = ph3c + ph3s

    NCH = NP // W
    NTILE = NP // 128
    in_maps = []
    for d in range(NCORES):
        xtf = np.zeros((C, NP), dtype=np.float32)
        xb = np.zeros((1, NP), dtype=bf)
        cl = np.zeros((128, NP), dtype=bf)
        sl = np.zeros((128, NP), dtype=bf)
        cd = np.zeros((128, NP), dtype=bf)
        sd = np.zeros((128, NP), dtype=bf)
        knf = np.zeros((NP, E), dtype=np.float32)
        for lj in range(GPD):
            gb = d * GPD + lj
            s, e_, cnt = starts[gb], starts[gb + 1], counts[gb]
            if cnt == 0:
                continue
            o = lj * slot
            xtf[:, o : o + cnt] = x[s:e_].T
            if has_bias:
                xb[0, o : o + cnt] = 1.0
            cl[:, o : o + cnt] = cphase[s:e_, 0:128].T.astype(bf)
            sl[:, o : o + cnt] = sphase[s:e_, 0:128].T.astype(bf)
            cd[0:64, o : o + cnt] = cphase[s:e_, 128:NT].T.astype(bf)
            cd[64:128, o : o + cnt] = cphase[s:e_, 128:NT].T.astype(bf)
            sd[0:64, o : o + cnt] = sphase[s:e_, 128:NT].T.astype(bf)
            sd[64:128, o : o + cnt] = sphase[s:e_, 128:NT].T.astype(bf)
            knf[o : o + cnt, :] = kfull[s:e_]
        # pre-chunk/transpose for contiguous per-partition DMA runs
        xt = np.ascontiguousarray(
            xtf.reshape(3, 128, NCH, W).transpose(2, 1, 0, 3)
        ).astype(bf)
        kn = np.ascontiguousarray(
            knf.reshape(NTILE, 128, E).transpose(1, 0, 2)
        ).astype(bf)
        m = {
            "xt": xt,
            "cl": cl,
            "sl": sl,
            "cd": cd,
            "sd": sd,
            "kn": kn,
            "wqa": wqa,
            "wva": wva,
            "wos": wos,
            "wosn": wosn,
        }
        if has_bias:
            m["xb"] = xb
        in_maps.append(m)

    want_trace = bool(int(os.environ.get("PLATCONV_TRACE", "0")))
    if want_trace:
        want_trace = _ensure_ntff_hook()
    res = run_bass_kernel_spmd(
        nc,
        in_maps,
        core_ids=list(range(NCORES)),
        trace=want_trace,
    )
    last_exec_time_ns = res.exec_time_ns
    global last_results
    last_results = res

    out = np.zeros((N, C), dtype=np.float32)
    for d in range(NCORES):
        ot = np.asarray(res.results[d]["outt"], dtype=np.float32).reshape(C, NP)
        for lj in range(GPD):
            gb = d * GPD + lj
            s, e_, cnt = starts[gb], starts[gb + 1], counts[gb]
            if cnt == 0:
                continue
            o = lj * slot
            out[s:e_] = ot[:, o : o + cnt].T
    out += bo[None, :]
    return out


# revision 47
# speedup vs baseline: 1.1378x; 1.0389x over previous
"""Trainium2 Bass kernel for nn_PlatonicConv (linear-attention GNN message passing).

Math (reference):
  q = rope(x@Wq + bq, phase);  k = rope(ones, phase);  v = x@Wv + bv
  phase[n, g, p] = pos[n, :] . freqs[g, 0, p, :]
  KV_b[g] = (1/AVG) * sum_{n in graph b} k[n,g,:] (x) v[n,g,:]
  out[n]  = concat_g( q'[n,g,:] @ KV_b[g] ) @ Wo + bo

Device formulation (per core, data-parallel over graphs; 8 graphs/core):
  host precomputes cos/sin of phase (feature-major, bf16) and node-major
  k = rope(ones) (unscaled; 1/AVG folded into Wo).  Per graph b:
    M_b = stack_rows(KV_b[g] @ Wo[g-rows]) : [384, 384]
    out[n] = q'[n] @ M_{b(n)}  (+ bo on host).
  q'/M_b rows use "A-order" over rope pairs t = g*16+p:
    rows   0:128 = E_t (even q dims), t=0..127     -> psum bank qE
    rows 128:256 = O_t (odd  q dims), t=0..127     -> psum bank qO
    rows 256:384 = [E_t | O_t], t=128..191         -> psum bank qEO2
  Rope is elementwise on full-width [128, W] tiles (ACT casts psum->sbuf
  bf16 first so DVE runs in 2x mode); the E2/O2 mix uses duplicated trig
  tiles plus half-partition sub/add.
  KV^T per graph: dense 4-group [128x128] blocks, all 3 chunks in ONE psum
  bank; 32x32 diagonal blocks copied (4 strided DVE copies) into a single
  pre-zeroed block-diagonal arena [128, 3*128] whose column halves give the
  even/odd-row stationaries for 6 M_b matmuls.
  Out: per (col-chunk, window) 3-matmul accumulation q0/q1/q2 against
  mbE/mbO/mbEO2 stationaries; drains casted bf16 and DMA'd per window.
  PE stream is software-pipelined: ... KV(j), Mb(j-1), out(j-2) ... so
  cross-engine handoffs (DVE arena copies, ACT mb casts) are hidden.

Self-contained: hardcodes shapes; shards/pads on host inside kernel().
"""

import math
import os
from contextlib import ExitStack

import ml_dtypes
import numpy as np

import concourse.bacc as bacc_mod
import concourse.bass as bass  # noqa: F401
import concourse.mybir as mybir
import concourse.tile as tile
from concourse.bass_utils import run_bass_kernel_spmd


def _ensure_ntff_hook():
    """Register the axon NTFF profile hook if the image's antenv lacks it."""
    try:
        import antenv.axon_hooks  # noqa: F401

        return True
    except ImportError:
        pass
    try:
        import sys
        import types

        import antenv
        from trn_agent_boot.trn_boot import _ntff_profile_via_ctypes

        mod = types.ModuleType("antenv.axon_hooks")
        _hook = [None]
        mod.set_axon_ntff_profile_hook = lambda h: _hook.__setitem__(0, h)
        mod.get_axon_ntff_profile_hook = lambda: _hook[0]
        sys.modules["antenv.axon_hooks"] = mod
        antenv.axon_hooks = mod
        mod.set_axon_ntff_profile_hook(
            _ntff_profile_via_ctypes("/opt/axon/libaxon_pjrt.so")
        )
        return True
    except Exception:
        return False


FP32 = mybir.dt.float32
BF16 = mybir.dt.bfloat16
FP8 = mybir.dt.float8e4
AF = mybir.ActivationFunctionType

N = 32768
C = 384
E = 384
G = 12
D = 32
P = 16
SD = 3
NUM_GRAPHS = 64
NCORES = 8
GPD = NUM_GRAPHS // NCORES  # graphs per device
AVG = float(N) / NUM_GRAPHS  # 512.0
NT = 192  # rope pairs = G*P
W = 512  # streaming window


def _a_order_cols():
    """perm such that A-order column r is original q-dim perm[r].

    r in [0,128):   E_t, t=r        (q dim g*32 + 2p,   g=t//16, p=t%16)
    r in [128,256): O_t, t=r-128    (q dim g*32 + 2p+1)
    r in [256,320): E_t, t=128+(r-256)
    r in [320,384): O_t, t=128+(r-320)
    """
    perm = np.empty(E, dtype=np.int64)
    for r in range(E):
        if r < 128:
            t, odd = r, 0
        elif r < 256:
            t, odd = r - 128, 1
        elif r < 320:
            t, odd = 128 + (r - 256), 0
        else:
            t, odd = 128 + (r - 320), 1
        perm[r] = (t // 16) * 32 + 2 * (t % 16) + odd
    return perm


_APERM = _a_order_cols()

_CACHE = {}


def _build(slot: int, has_bias: bool):
    debug = bool(int(os.environ.get("PLATCONV_DEBUG", "0")))
    key = (slot, has_bias, debug)
    if key in _CACHE:
        return _CACHE[key]

    NP = GPD * slot
    NTILE = NP // 128
    TPS = slot // 128
    NCH = NP // W
    assert NP % W == 0
    HNP = NP // 2  # trig half split for earlier availability

    nc = bacc_mod.Bacc()

    CA = C + 1 if has_bias else C

    # x, pre-chunked/transposed on host for contiguous per-partition runs
    xt_d = nc.declare_dram_parameter("xt", [NCH, 128, 3, W], BF16, isOutput=False)
    if has_bias:
        xb_d = nc.declare_dram_parameter("xb", [1, NP], BF16, isOutput=False)
    # trig, feature-major: rows 0:128 = t<128; dup tiles hold t 128:192 twice
    cl_d = nc.declare_dram_parameter("cl", [128, NP], BF16, isOutput=False)
    sl_d = nc.declare_dram_parameter("sl", [128, NP], BF16, isOutput=False)
    cd_d = nc.declare_dram_parameter("cd", [128, NP], BF16, isOutput=False)
    sd_d = nc.declare_dram_parameter("sd", [128, NP], BF16, isOutput=False)
    # k, node-major pre-transposed: kn[p, t, e] = k[t*128+p, e]
    kn_d = nc.declare_dram_parameter("kn", [128, NP // 128, E], BF16, isOutput=False)
    wqa_d = nc.declare_dram_parameter("wqa", [CA, E], BF16, isOutput=False)
    wva_d = nc.declare_dram_parameter("wva", [CA, E], BF16, isOutput=False)
    wos_d = nc.declare_dram_parameter("wos", [E, C], BF16, isOutput=False)
    wosn_d = nc.declare_dram_parameter("wosn", [128, C], BF16, isOutput=False)
    out_d = nc.declare_dram_parameter("outt", [3, 128, NP], BF16, isOutput=True)
    if debug:
        dbg_q = nc.declare_dram_parameter("dbgq", [4, 128, NP], BF16, isOutput=True)
        dbg_v = nc.declare_dram_parameter("dbgv", [2, 128, NP // 128, E], BF16, isOutput=True)
        dbg_mb = nc.declare_dram_parameter("dbgmb", [GPD, 4, 128, C], BF16, isOutput=True)
        dbg_ar = nc.declare_dram_parameter("dbgar", [GPD, 128, 384], BF16, isOutput=True)

    with ExitStack() as ctx:
        tc = ctx.enter_context(tile.TileContext(nc))

        consts = ctx.enter_context(tc.tile_pool(name="consts", bufs=1))
        xtp = ctx.enter_context(tc.tile_pool(name="xtp", bufs=4))
        qsb = ctx.enter_context(tc.tile_pool(name="qsb", bufs=2))
        big = ctx.enter_context(tc.tile_pool(name="big", bufs=1))
        mbp = ctx.enter_context(tc.tile_pool(name="mbp", bufs=3))
        outp = ctx.enter_context(tc.tile_pool(name="outp", bufs=3))
        psum = ctx.enter_context(tc.tile_pool(name="psum", bufs=1, space="PSUM"))

        def pbank(tag):
            """One full PSUM bank ([128, 512] f32)."""
            return psum.tile([128, W], FP32, tag=tag, name=tag)

        # ---- weight loads; xt chunks 0/1 issued between wq and wv so the
        # first Q chain starts as early as possible ----
        wq_t = consts.tile([128, 3, E], BF16, tag="wq")
        nc.sync.dma_start(
            wq_t[:], wqa_d[0:C, :].rearrange("(b p) e -> p b e", p=128)
        )
        wv_t = consts.tile([128, 3, E], BF16, tag="wv")
        if has_bias:
            wqb = consts.tile([1, E], BF16, tag="wqb")
            nc.sync.dma_start(wqb[:], wqa_d[C : C + 1, :])
            wvb = consts.tile([1, E], BF16, tag="wvb")
            nc.sync.dma_start(wvb[:], wva_d[C : C + 1, :])

        # ---- persistent SBUF tensors ----
        q0 = big.tile([128, NP], BF16, tag="q0")  # E' rows t<128
        q1 = big.tile([128, NP], BF16, tag="q1")  # O' rows t<128
        # EO2 rope products, recombined on PE via mb2/mb2x stationaries:
        q2m1 = big.tile([128, NP], BF16, tag="q2m1")  # [E2*c2 ; O2*c2]
        q2m2 = big.tile([128, NP], BF16, tag="q2m2")  # [E2*s2 ; O2*s2]
        v_sb = big.tile([128, NTILE, E], BF16, tag="v_sb")
        k_sb = big.tile([128, NTILE, E], BF16, tag="k_sb")
        clf = big.tile([128, NP], BF16, tag="clf")
        slf = big.tile([128, NP], BF16, tag="slf")
        cdf = big.tile([128, NP], BF16, tag="cdf")  # [c2;c2] dup rows t>=128
        sdf = big.tile([128, NP], BF16, tag="sdf")  # [s2;s2]
        wos_t = consts.tile([128, 3, C], BF16, tag="wos")  # quad q rows
        wosn_t = consts.tile([128, C], BF16, tag="wosn")  # -wos, quad 2 rows

        # block-diag arena: cols [q*128 + eo*64 + m*16 + p]; zero once per set
        arenas = []
        for s in range(2):
            a = big.tile([128, 3 * 128], BF16, tag=f"arena{s}")
            nc.vector.memset(a[:], 0.0)
            arenas.append(a)

        # DMA issue schedule (sync queue is FIFO; order = priority).
        # xt prefetches are issued first in each iteration (see loop); trig
        # tiles stream in quarters interleaved with the xt stream so supply
        # tracks demand; kn/wos ride at the end.  EO2 rope muls for chunk ch
        # are emitted in iteration ch+1, so cdf/sdf may lag cl/sl slightly.
        QNP = NP // 4

        def quarter(dst, src, qi):
            nc.sync.dma_start(
                dst[:, qi * QNP : (qi + 1) * QNP], src[:, qi * QNP : (qi + 1) * QNP]
            )

        def extra_dmas(ch):
            if ch == 0:
                quarter(cdf, cd_d, 0)
                quarter(sdf, sd_d, 0)
            elif ch == 1:
                quarter(clf, cl_d, 1)
                quarter(slf, sl_d, 1)
            elif ch == 2:
                quarter(cdf, cd_d, 1)
                quarter(sdf, sd_d, 1)
            elif ch == 3:
                quarter(clf, cl_d, 2)
                quarter(slf, sl_d, 2)
            elif ch == 4:
                quarter(cdf, cd_d, 2)
                quarter(sdf, sd_d, 2)
            elif ch == 5:
                quarter(clf, cl_d, 3)
                quarter(slf, sl_d, 3)
                quarter(cdf, cd_d, 3)
                quarter(sdf, sd_d, 3)
            elif ch == 6:
                nc.sync.dma_start(
                    k_sb[:, 0 : NTILE // 2, :], kn_d[:, 0 : NTILE // 2, :]
                )
            elif ch == 7:
                nc.sync.dma_start(
                    k_sb[:, NTILE // 2 : NTILE, :], kn_d[:, NTILE // 2 : NTILE, :]
                )
            elif ch == 8:
                nc.sync.dma_start(
                    wos_t[:], wos_d[:].rearrange("(b p) e -> p b e", p=128)
                )
                nc.sync.dma_start(wosn_t[:], wosn_d[:])

        # ------------------------------------------------------------------
        # Phase A: Q/V projections + rope, per chunk of W nodes
        # psum tags: even chunk qE/qO/q2 = A0,A1,A2; odd = A3,A4,A5; v = A6,A7
        # ------------------------------------------------------------------
        def xt_load(ch):
            n0 = ch * W
            t = xtp.tile([128, 3, W], BF16, tag="xt")
            nc.sync.dma_start(t[:], xt_d[ch])
            if has_bias:
                tb = xtp.tile([1, W], BF16, tag="xtb")
                nc.sync.dma_start(tb[:], xb_d[:, n0 : n0 + W])
                return (t, tb)
            return (t, None)

        xt_tiles = {0: xt_load(0), 1: xt_load(1)}
        nc.sync.dma_start(
            wv_t[:], wva_d[0:C, :].rearrange("(b p) e -> p b e", p=128)
        )
        quarter(clf, cl_d, 0)
        quarter(slf, sl_d, 0)

        eo2_pending = []

        def emit_eo2(ch, q2s):
            n0 = ch * W
            nc.gpsimd.tensor_mul(
                q2m1[:, n0 : n0 + W], q2s[:], cdf[:, n0 : n0 + W]
            )
            nc.gpsimd.tensor_mul(
                q2m2[:, n0 : n0 + W], q2s[:], sdf[:, n0 : n0 + W]
            )

        for ch in range(NCH):
            n0 = ch * W
            par = ch % 2
            xt_c, xt_b = xt_tiles.pop(ch)
            if ch + 2 < NCH:
                xt_tiles[ch + 2] = xt_load(ch + 2)
            extra_dmas(ch)

            # Q banks interleaved with V subtile chains; ACT drains emitted
            # in readiness order so the ACT FIFO never head-of-line blocks.
            qEs = qsb.tile([128, W], BF16, tag="qEs")
            qOs = qsb.tile([128, W], BF16, tag="qOs")
            q2s = qsb.tile([128, W], BF16, tag="q2s")
            qcast = [qEs, qOs, q2s]
            qbank = [pbank(f"A{3 * par + i}") for i in range(3)]

            def q_chain(bk):
                c0 = 128 * bk
                for ki in range(3):
                    nc.tensor.matmul(
                        qbank[bk][:],
                        wq_t[:, ki, c0 : c0 + 128],
                        xt_c[:, ki, :],
                        start=(ki == 0),
                        stop=(ki == 2 and not has_bias),
                    )
                if has_bias:
                    nc.tensor.matmul(
                        qbank[bk][:],
                        wqb[:, c0 : c0 + 128],
                        xt_b[:],
                        start=False,
                        stop=True,
                    )
                nc.scalar.activation(qcast[bk][:], qbank[bk][:], AF.Copy)

            def v_chain(sub):
                ti = ch * (W // 128) + sub
                f0 = sub * 128
                vps = pbank(f"A{6 + sub % 2}")
                for ki in range(3):
                    nc.tensor.matmul(
                        vps[:, 0:E],
                        xt_c[:, ki, f0 : f0 + 128],
                        wv_t[:, ki, :],
                        start=(ki == 0),
                        stop=(ki == 2 and not has_bias),
                    )
                if has_bias:
                    nc.tensor.matmul(
                        vps[:, 0:E],
                        xt_b[:, f0 : f0 + 128],
                        wvb[:],
                        start=False,
                        stop=True,
                    )
                if sub == 3:
                    nc.vector.tensor_copy(v_sb[:, ti, :], vps[:, 0:E])
                else:
                    nc.scalar.activation(v_sb[:, ti, :], vps[:, 0:E], AF.Copy)

            q_chain(0)
            v_chain(0)
            q_chain(1)
            v_chain(1)
            q_chain(2)
            v_chain(2)
            v_chain(3)

            # rope (DVE, all [128, W], all operands partition-aligned)
            cw = lambda t: t[:, n0 : n0 + W]
            ta = qsb.tile([128, W], BF16, tag="ta")
            tb = qsb.tile([128, W], BF16, tag="tb")
            nc.vector.tensor_mul(ta[:], qEs[:], cw(clf))
            nc.vector.tensor_mul(tb[:], qOs[:], cw(slf))
            nc.vector.tensor_sub(cw(q0), ta[:], tb[:])
            nc.vector.tensor_mul(ta[:], qEs[:], cw(slf))
            nc.vector.tensor_mul(tb[:], qOs[:], cw(clf))
            nc.vector.tensor_add(cw(q1), ta[:], tb[:])
            # EO2 products on Pool (SBUF-only), deferred one iteration so
            # the cdf/sdf DMAs can be emitted after the xt stream starts;
            # PE recombines E2'/O2' via the mb2x stationary.
            for pch, pq2s in eo2_pending:
                emit_eo2(pch, pq2s)
            eo2_pending = [(ch, q2s)]

        # ------------------------------------------------------------------
        # Phase B per graph: KV (1 bank), arena copies, M_b (3 banks), out.
        # PE stream: KV(j), Mb(j-1), out(j-2) -- handoffs hidden.
        # psum tags: kvt = A0/A1 (j%2), mbE/mbO/mb2 = A2,A3,A4, out = A5/A6
        # ------------------------------------------------------------------
        mb_tiles = [None] * GPD  # sbuf stationaries per graph
        oti = 0

        def kv_stage(j, bank="A0"):
            t0 = j * TPS
            kvt = pbank(bank)
            for cchunk in range(3):
                for tt in range(TPS):
                    nc.tensor.matmul(
                        kvt[:, 128 * cchunk : 128 * (cchunk + 1)],
                        v_sb[:, t0 + tt, 128 * cchunk : 128 * (cchunk + 1)],
                        k_sb[:, t0 + tt, 128 * cchunk : 128 * (cchunk + 1)],
                        start=(tt == 0),
                        stop=(tt == TPS - 1),
                    )
            # arena copies: one DVE copy per m (3 groups each); k columns are
            # host-ordered d' = eo*16 + p, so both src and dst iterate
            # (q, eo, p) with contiguous 16-element p-runs.
            A = arenas[j % 2]
            for m in range(4):
                r0 = 32 * m
                src = kvt[r0 : r0 + 32, 0 : 3 * 128].rearrange(
                    "e (q d) -> e q d", q=3
                )[:, :, r0 : r0 + 32].rearrange("e q (eo p) -> e q eo p", eo=2)
                dst = A[r0 : r0 + 32, :].rearrange(
                    "e (q eo hp p) -> e q eo p hp", q=3, eo=2, hp=4
                )[:, :, :, :, m]
                nc.vector.tensor_copy(dst, src)
            return kvt

        def mb_stage(j):
            A = arenas[j % 2]
            mbps = [pbank("A2"), pbank("A3"), pbank("A4")]
            # bank r: matmul quads with E-sel (r=0), O-sel (r=1), EO2 (r=2)
            for quad, (bank, colsel, mpos) in enumerate(
                (
                    (0, slice(0, 64), 0),
                    (0, slice(128, 192), 64),
                    (1, slice(64, 128), 0),
                    (1, slice(192, 256), 64),
                    (2, slice(256, 320), 0),
                    (2, slice(320, 384), 64),
                )
            ):
                q = colsel.start // 128
                nc.tensor.matmul(
                    mbps[bank][mpos : mpos + 64, 0:C],
                    A[:, colsel],
                    wos_t[:, q, :],
                    start=True,
                    stop=True,
                    tile_position=(0, mpos),
                )
            # mb2x = [M[O2-rows] ; -M[E2-rows]] for the q2m2 accumulation term
            mb2xps = pbank("A7")
            nc.tensor.matmul(
                mb2xps[0:64, 0:C],
                A[:, 320:384],
                wos_t[:, 2, :],
                start=True,
                stop=True,
                tile_position=(0, 0),
            )
            nc.tensor.matmul(
                mb2xps[64:128, 0:C],
                A[:, 256:320],
                wosn_t[:],
                start=True,
                stop=True,
                tile_position=(0, 64),
            )
            # casts to sbuf stationaries: 2 ACT + 2 DVE
            mb0 = mbp.tile([128, C], BF16, tag="mb0")
            mb1 = mbp.tile([128, C], BF16, tag="mb1")
            mb2 = mbp.tile([128, C], BF16, tag="mb2")
            mb2x = mbp.tile([128, C], BF16, tag="mb2x")
            nc.scalar.activation(mb0[:], mbps[0][:, 0:C], AF.Copy)
            nc.scalar.activation(mb1[:], mbps[1][:, 0:C], AF.Copy)
            nc.vector.tensor_copy(mb2[:], mbps[2][:, 0:C])
            nc.vector.tensor_copy(mb2x[:], mb2xps[:, 0:C])
            mb_tiles[j] = (mb0, mb1, mb2, mb2x)
            if debug:
                for i, t in enumerate((mb0, mb1, mb2, mb2x)):
                    nc.sync.dma_start(dbg_mb[j, i], t[:])
                nc.sync.dma_start(dbg_ar[j], A[:])

        def out_stage(j):
            nonlocal oti
            mb0, mb1, mb2, mb2x = mb_tiles[j]
            slot0 = j * slot
            wins = []
            o = 0
            while o < slot:
                w = min(W, slot - o)
                wins.append((slot0 + o, w))
                o += w
            for w0, w in wins:
                ost = outp.tile([128, 3, W], BF16, tag="ost")
                for cch in range(3):
                    cc = slice(128 * cch, 128 * (cch + 1))
                    ot = pbank(("A5", "A6", "A1")[oti % 3])
                    nc.tensor.matmul(
                        ot[:, :w], mb0[:, cc], q0[:, w0 : w0 + w],
                        start=True, stop=False,
                    )
                    nc.tensor.matmul(
                        ot[:, :w], mb1[:, cc], q1[:, w0 : w0 + w],
                        start=False, stop=False,
                    )
                    nc.tensor.matmul(
                        ot[:, :w], mb2[:, cc], q2m1[:, w0 : w0 + w],
                        start=False, stop=False,
                    )
                    nc.tensor.matmul(
                        ot[:, :w], mb2x[:, cc], q2m2[:, w0 : w0 + w],
                        start=False, stop=True,
                    )
                    # drains: alternate ACT/DVE (Pool cannot read PSUM)
                    if oti % 2 == 0:
                        nc.scalar.activation(ost[:, cch, :w], ot[:, :w], AF.Copy)
                    else:
                        nc.vector.tensor_copy(ost[:, cch, :w], ot[:, :w])
                    oti += 1
                    nc.sync.dma_start(
                        out_d[cch, :, w0 : w0 + w], ost[:, cch, :w]
                    )

        for pch, pq2s in eo2_pending:
            emit_eo2(pch, pq2s)
        eo2_pending = []

        # software pipeline (out lags by 3 so mb casts are never waited on);
        # kv(0) borrows an out bank so kv(1) needn't wait for arena copies
        kv_stage(0, bank="A6")
        kv_stage(1)
        mb_stage(0)
        kv_stage(2)
        mb_stage(1)
        for j in range(3, GPD):
            kv_stage(j)
            mb_stage(j - 1)
            out_stage(j - 3)
        mb_stage(GPD - 1)
        out_stage(GPD - 3)
        out_stage(GPD - 2)
        out_stage(GPD - 1)

        if debug:
            for i, t in enumerate((q0, q1, q2m1, q2m2)):
                nc.sync.dma_start(dbg_q[i], t[:])
            nc.sync.dma_start(dbg_v[0], v_sb[:])
            nc.sync.dma_start(dbg_v[1], k_sb[:])

    nc.compile()

    _CACHE[key] = (nc, NP)
    return nc, NP


last_exec_time_ns = None
last_results = None


def kernel(x, pos, batch, Wq, bq, Wv, bv, Wo, bo, freqs):
    global last_exec_time_ns
    x = np.asarray(x, dtype=np.float32)
    pos = np.asarray(pos, dtype=np.float32)
    batch = np.asarray(batch).astype(np.int64)
    Wq = np.asarray(Wq, dtype=np.float32)
    bq = np.asarray(bq, dtype=np.float32)
    Wv = np.asarray(Wv, dtype=np.float32)
    bv = np.asarray(bv, dtype=np.float32)
    Wo = np.asarray(Wo, dtype=np.float32)
    bo = np.asarray(bo, dtype=np.float32)
    freqs = np.asarray(freqs, dtype=np.float32)

    counts = np.bincount(batch, minlength=NUM_GRAPHS)
    starts = np.concatenate([[0], np.cumsum(counts)])
    slot = max(640, int(math.ceil(counts.max() / 128.0)) * 128)
    has_bias = bool(np.any(bq) or np.any(bv))

    nc, NP = _build(slot, has_bias)

    WqA = Wq[:, _APERM]
    bqA = bq[_APERM]
    CA = C + 1 if has_bias else C
    bf = ml_dtypes.bfloat16

    wqa = np.zeros((CA, E), dtype=bf)
    wqa[:C] = WqA.astype(bf)
    wva = np.zeros((CA, E), dtype=bf)
    wva[:C] = Wv.astype(bf)
    if has_bias:
        wqa[C] = bqA.astype(bf)
        wva[C] = bv.astype(bf)
    wosf = Wo * (1.0 / AVG)
    wos = wosf.astype(bf)
    wosn = (-wosf[256:384]).astype(bf)

    # phase & trig on host (t = g*16+p, g-major)
    fr = freqs.reshape(NT, SD)
    phase = pos @ fr.T  # [N, 192] float32
    cphase = np.cos(phase)
    sphase = np.sin(phase)
    # k node-major, eo-grouped cols: col g*32 + eo*16 + p, eo=0: c-s, eo=1: c+s
    kfull = np.empty((len(x), E), dtype=np.float32)
    k3 = kfull.reshape(len(x), G, 2, P)
    ph3c = cphase.reshape(len(x), G, P)
    ph3s = sphase.reshape(len(x), G, P)
    k3[:, :, 0, :] = ph3c - ph3s
    k3[:, :, 1, :] = ph3c + ph3s

    NCH = NP // W
    NTILE = NP // 128
    in_maps = []
    for d in range(NCORES):
        xtf = np.zeros((C, NP), dtype=np.float32)
        xb = np.zeros((1, NP), dtype=bf)
        cl = np.zeros((128, NP), dtype=bf)
        sl = np.zeros((128, NP), dtype=bf)
        cd = np.zeros((128, NP), dtype=bf)
        sd = np.zeros((128, NP), dtype=bf)
        knf = np.zeros((NP, E), dtype=np.float32)
        for lj in range(GPD):
            gb = d * GPD + lj
            s, e_, cnt = starts[gb], starts[gb + 1], counts[gb]
            if cnt == 0:
                continue
            o = lj * slot
            xtf[:, o : o + cnt] = x[s:e_].T
            if has_bias:
                xb[0, o : o + cnt] = 1.0
            cl[:, o : o + cnt] = cphase[s:e_, 0:128].T.astype(bf)
            sl[:, o : o + cnt] = sphase[s:e_, 0:128].T.astype(bf)
            cd[0:64, o : o + cnt] = cphase[s:e_, 128:NT].T.astype(bf)
            cd[64:128, o : o + cnt] = cphase[s:e_, 128:NT].T.astype(bf)
            sd[0:64, o : o + cnt] = sphase[s:e_, 128:NT].T.astype(bf)
            sd[64:128, o : o + cnt] = sphase[s:e_, 128:NT].T.astype(bf)
            knf[o : o + cnt, :] = kfull[s:e_]
        # pre-chunk/transpose for contiguous per-partition DMA runs
        xt = np.ascontiguousarray(
            xtf.reshape(3, 128, NCH, W).transpose(2, 1, 0, 3)
        ).astype(bf)
        kn = np.ascontiguousarray(
            knf.reshape(NTILE, 128, E).transpose(1, 0, 2)
        ).astype(bf)
        m = {
            "xt": xt,
            "cl": cl,
            "sl": sl,
            "cd": cd,
            "sd": sd,
            "kn": kn,
            "wqa": wqa,
            "wva": wva,
            "wos": wos,
            "wosn": wosn,
        }
        if has_bias:
            m["xb"] = xb
        in_maps.append(m)

    want_trace = bool(int(os.environ.get("PLATCONV_TRACE", "0")))
    if want_trace:
        want_trace = _ensure_ntff_hook()
    res = run_bass_kernel_spmd(
        nc,
        in_maps,
        core_ids=list(range(NCORES)),
        trace=want_trace,
    )
    last_exec_time_ns = res.exec_time_ns
    global last_results
    last_results = res

    out = np.zeros((N, C), dtype=np.float32)
    for d in range(NCORES):
        ot = np.asarray(res.results[d]["outt"], dtype=np.float32).reshape(C, NP)
        for lj in range(GPD):
            gb = d * GPD + lj
            s, e_, cnt = starts[gb], starts[gb + 1], counts[gb]
            if cnt == 0:
                continue
            o = lj * slot
            out[s:e_] = ot[:, o : o + cnt].T
    out += bo[None, :]
    return out
